# revision 1
# baseline (speedup 1.0000x reference)
"""Griffin recurrence Trainium2 kernel.

Sharding: 8 cores = 4 batches x 2 channel-halves (192 channels each).
Layout on device: [channels, seq]. The projection matmul runs on the PE in
float32r (fp32 rounded to 11 mantissa bits, full PE rate); the chunked
log-space scan of the reference is replicated exactly with masked
tensor_tensor_scan ops on the DVE plus ACT transcendentals.

Per core:
  inputs  xT  [2048, seq]  x[b].T, fp32r-rounded
          wT  [2048, 640]  packed W slice (see _pack_w), fp32r-rounded
          db0 [128, 1]     decay bias for channels 0..127 of this half
          db1 [128, 1]     decay bias for channels 128..191 (rows 0..63), zeros after
  output  out [192, seq]   states, channel-major
"""

import sys

sys.path.insert(0, "/opt/trn_rl_repo")

from contextlib import ExitStack

import numpy as np

from concourse import bacc, mybir, tile
from concourse.bass_utils import run_bass_kernel_spmd

f32 = mybir.dt.float32
f32r = mybir.dt.float32r
AF = mybir.ActivationFunctionType
ALU = mybir.AluOpType

D_MODEL = 2048
D_REC = 384
CHUNK = 64
NCORE = 8
CH = 192  # channels per core
SCW = 256  # seq-chunk width processed per pipeline step
NK = D_MODEL // 128  # 16 k-tiles
NT = 5  # M-tiles per core (640 = 5*128 packed W rows)
EPS_LOG = 1e-10

_built = {}


def _round_fp32r(a: np.ndarray) -> np.ndarray:
    """Round fp32 to fp32r (11 mantissa bits, low 12 bits zero), RNE."""
    u = np.ascontiguousarray(a, dtype=np.float32).view(np.uint32)
    rem = u & np.uint32(0xFFF)
    keep = u & np.uint32(0xFFFFF000)
    lsb = (u >> np.uint32(12)) & np.uint32(1)
    up = (rem > 0x800) | ((rem == 0x800) & (lsb == 1))
    return (keep + (up.astype(np.uint32) << np.uint32(12))).view(np.float32)


def _emit(tc, nc, xT, wT, db0, db1, out, seq):
    nsc = seq // SCW
    nch = seq // CHUNK  # chunks per sequence
    cpc = SCW // CHUNK  # chunks per seq-chunk (4)

    with ExitStack() as ctx:
        const = ctx.enter_context(tc.tile_pool(name="const", bufs=1))
        persist = ctx.enter_context(tc.tile_pool(name="persist", bufs=1))
        xp = ctx.enter_context(tc.tile_pool(name="xp", bufs=3))
        pp = ctx.enter_context(tc.tile_pool(name="pp", bufs=1, space="PSUM"))
        pv = ctx.enter_context(tc.tile_pool(name="pv", bufs=2, space="PSUM"))
        wk = ctx.enter_context(tc.tile_pool(name="wk", bufs=2))
        sm = ctx.enter_context(tc.tile_pool(name="sm", bufs=1))

        # constants (x-stream owns the SP queue; everything else goes on the
        # scalar-engine HWDGE queue so the 2 MB x DMAs are never stuck behind
        # small transfers)
        mask = const.tile([128, SCW], f32, tag="mask")
        nc.vector.memset(mask[:], 1.0)
        for c in range(cpc):
            nc.vector.memset(mask[:, c * CHUNK : c * CHUNK + 1], 0.0)
        ones = const.tile([128, nch], f32, tag="ones")
        nc.vector.memset(ones[:], 1.0)
        db0_t = const.tile([128, 1], f32, tag="db0")
        nc.scalar.dma_start(db0_t[:], db0[:])
        db1_t = const.tile([128, 1], f32, tag="db1")
        nc.scalar.dma_start(db1_t[:], db1[:])
        wt = []
        for k in range(NK):
            w = const.tile([128, NT * 128], f32r, tag=f"wt{k}")
            nc.scalar.dma_start(w[:], wT[k * 128 : (k + 1) * 128, :])
            wt.append(w)

        # persistent per-chunk state (alpha: channels 0..127, beta: 128..191)
        cdA = persist.tile([128, seq], f32, tag="cdA")
        itA = persist.tile([128, seq], f32, tag="itA")
        cdB = persist.tile([64, seq], f32, tag="cdB")
        itB = persist.tile([64, seq], f32, tag="itB")

        groups = (
            ("A", 128, cdA, itA, db0_t, 0),
            ("B", 64, cdB, itB, db1_t, 128),
        )

        for sc in range(nsc):
            s0 = sc * SCW
            # all 16 k-tiles of this seq-chunk in one 2 MB DMA
            xall = xp.tile([128, NK * SCW], f32r, tag="xall")
            nc.sync.dma_start(
                xall[:].rearrange("p (k s) -> p k s", k=NK),
                xT[:, s0 : s0 + SCW].rearrange("(k p) s -> p k s", p=128),
            )
            ps = []
            for t in range(NT):
                pool = pv if t in (2, 4) else pp
                p = pool.tile([128, SCW], f32, tag=f"ps{t}")
                for k in range(NK):
                    nc.tensor.matmul(
                        p[:],
                        wt[k][:, t * 128 : (t + 1) * 128],
                        xall[:, k * SCW : (k + 1) * SCW],
                        start=(k == 0),
                        stop=(k == NK - 1),
                    )
                ps.append(p)

            # nonlinearities straight out of PSUM (only Sigmoid/Ln/Exp/Sqrt on
            # ACT so the function tables never swap); v stays in PSUM (pv pool,
            # double-buffered) until the iv multiply consumes it on the DVE
            aA = wk.tile([128, SCW], f32, tag="aA")
            nc.scalar.activation(aA[:], ps[0][:], AF.Sigmoid, bias=db0_t[:])
            iA = wk.tile([128, SCW], f32, tag="iA")
            nc.scalar.activation(iA[:], ps[1][:], AF.Sigmoid)
            ab = wk.tile([128, SCW], f32, tag="ab")
            nc.scalar.activation(ab[:], ps[3][:], AF.Sigmoid, bias=db1_t[:])
            iB = wk.tile([64, SCW], f32, tag="iB")
            nc.sync.dma_start(iB[:], ab[64:128, :])  # realign beta i to partitions 0..63

            for name, pg, cd_all, it_all, _db, _orow in groups:
                if name == "A":
                    a_ap, i_ap, v_ap = aA[:], iA[:], ps[2][:]
                else:
                    a_ap, i_ap, v_ap = ab[0:64, :], iB[:], ps[4][0:64, :]
                cds = cd_all[:, s0 : s0 + SCW]
                la = wk.tile([pg, SCW], f32, tag="la")
                nc.scalar.activation(la[:], a_ap, AF.Ln)
                sla = wk.tile([pg, SCW], f32, tag="sla")
                nc.vector.tensor_tensor_scan(
                    sla[:], mask[0:pg, :], la[:], 0.0, ALU.mult, ALU.add
                )
                nc.scalar.activation(cds, sla[:], AF.Exp)
                a2 = wk.tile([pg, SCW], f32, tag="a2")
                nc.vector.tensor_mul(a2[:], a_ap, a_ap)
                sqt = wk.tile([pg, SCW], f32, tag="sqt")
                nc.scalar.activation(sqt[:], a2[:], AF.Sqrt, bias=1.0, scale=-1.0)
                iv = wk.tile([pg, SCW], f32, tag="iv")
                nc.vector.tensor_mul(iv[:], i_ap, v_ap)
                u = wk.tile([pg, SCW], f32, tag="u")
                nc.vector.tensor_mul(u[:], sqt[:], iv[:])
                cdc = wk.tile([pg, SCW], f32, tag="cdc")
                nc.vector.tensor_scalar_max(cdc[:], cds, EPS_LOG)
                rc = wk.tile([pg, SCW], f32, tag="rc")
                nc.vector.reciprocal(rc[:], cdc[:])
                w_ = wk.tile([pg, SCW], f32, tag="w_")
                nc.vector.tensor_mul(w_[:], u[:], rc[:])
                sw = wk.tile([pg, SCW], f32, tag="sw")
                nc.vector.tensor_tensor_scan(
                    sw[:], mask[0:pg, :], w_[:], 0.0, ALU.mult, ALU.add
                )
                nc.vector.tensor_mul(it_all[:, s0 : s0 + SCW], cds, sw[:])

        # cross-chunk scan per group
        incs = {}
        for name, pg, cd_all, it_all, _db, _orow in groups:
            cdb = sm.tile([pg, nch], f32, tag=f"cdb{name}")
            nc.vector.tensor_copy(cdb[:], cd_all[:, CHUNK - 1 :: CHUNK])
            itb = sm.tile([pg, nch], f32, tag=f"itb{name}")
            nc.vector.tensor_copy(itb[:], it_all[:, CHUNK - 1 :: CHUNK])
            dc = sm.tile([pg, nch], f32, tag=f"dc{name}")
            nc.vector.tensor_scalar_max(dc[:], cdb[:], EPS_LOG)
            ld = sm.tile([pg, nch], f32, tag=f"ld{name}")
            nc.scalar.activation(ld[:], dc[:], AF.Ln)
            cl = sm.tile([pg, nch], f32, tag=f"cl{name}")
            nc.vector.tensor_tensor_scan(
                cl[:], ones[0:pg, :], ld[:], 0.0, ALU.mult, ALU.add
            )
            CD = sm.tile([pg, nch], f32, tag=f"CD{name}")
            nc.scalar.activation(CD[:], cl[:], AF.Exp)
            CDc = sm.tile([pg, nch], f32, tag=f"CDc{name}")
            nc.vector.tensor_scalar_max(CDc[:], CD[:], EPS_LOG)
            rr = sm.tile([pg, nch], f32, tag=f"rr{name}")
            nc.vector.reciprocal(rr[:], CDc[:])
            terms = sm.tile([pg, nch], f32, tag=f"terms{name}")
            nc.vector.tensor_mul(terms[:], itb[:], rr[:])
            CW = sm.tile([pg, nch], f32, tag=f"CW{name}")
            nc.vector.tensor_tensor_scan(
                CW[:], ones[0:pg, :], terms[:], 0.0, ALU.mult, ALU.add
            )
            icd = sm.tile([pg, nch], f32, tag=f"icd{name}")
            nc.vector.memset(icd[:, 0:1], 1.0)
            nc.vector.tensor_copy(icd[:, 1:nch], CD[:, 0 : nch - 1])
            icw = sm.tile([pg, nch], f32, tag=f"icw{name}")
            nc.vector.memset(icw[:, 0:1], 0.0)
            nc.vector.tensor_copy(icw[:, 1:nch], CW[:, 0 : nch - 1])
            inc = sm.tile([pg, nch], f32, tag=f"inc{name}")
            nc.vector.tensor_mul(inc[:], icd[:], icw[:])
            incs[name] = inc

        # combine and write out
        for sc in range(nsc):
            s0 = sc * SCW
            for name, pg, cd_all, it_all, _db, orow in groups:
                inc = incs[name]
                ob = wk.tile([pg, SCW], f32, tag=f"ob{name}")
                for c4 in range(cpc):
                    c = sc * cpc + c4
                    t0 = c * CHUNK
                    nc.vector.scalar_tensor_tensor(
                        ob[:, c4 * CHUNK : (c4 + 1) * CHUNK],
                        cd_all[:, t0 : t0 + CHUNK],
                        inc[:, c : c + 1],
                        it_all[:, t0 : t0 + CHUNK],
                        ALU.mult,
                        ALU.add,
                    )
                nc.sync.dma_start(out[orow : orow + pg, s0 : s0 + SCW], ob[:])


def _build(seq):
    if seq in _built:
        return _built[seq]
    nc = bacc.Bacc(
        "TRN2", target_bir_lowering=False, debug=False, num_devices=NCORE
    )
    xT = nc.dram_tensor("xT", [D_MODEL, seq], f32r, kind="ExternalInput").ap()
    wT = nc.dram_tensor("wT", [D_MODEL, NT * 128], f32r, kind="ExternalInput").ap()
    db0 = nc.dram_tensor("db0", [128, 1], f32, kind="ExternalInput").ap()
    db1 = nc.dram_tensor("db1", [128, 1], f32, kind="ExternalInput").ap()
    out = nc.dram_tensor("out", [CH, seq], f32, kind="ExternalOutput").ap()
    with tile.TileContext(nc) as tc:
        _emit(tc, nc, xT, wT, db0, db1, out, seq)
    nc.compile()
    _built[seq] = nc
    return nc


def _pack_w(W, h):
    """Pack this half's W rows into 640 rows of 5 M-tiles.

    t0 = a[0:128], t1 = i[0:128], t2 = v[0:128],
    t3 = [a[128:192]; i[128:192]], t4 = [v[128:192]; zeros]."""
    c0 = h * CH
    z = np.zeros((64, W.shape[1]), np.float32)
    return np.concatenate(
        [
            W[c0 : c0 + 128],
            W[D_REC + c0 : D_REC + c0 + 128],
            W[2 * D_REC + c0 : 2 * D_REC + c0 + 128],
            W[c0 + 128 : c0 + 192],
            W[D_REC + c0 + 128 : D_REC + c0 + 192],
            W[2 * D_REC + c0 + 128 : 2 * D_REC + c0 + 192],
            z,
        ],
        axis=0,
    )


def _in_maps(x, W, db):
    maps = []
    xTs = {}
    for core in range(NCORE):
        b, h = core // 2, core % 2
        if b not in xTs:
            xTs[b] = _round_fp32r(np.ascontiguousarray(x[b].T))
        c0 = h * CH
        wTc = _round_fp32r(np.ascontiguousarray(_pack_w(W, h).T))
        db0v = np.ascontiguousarray(db[c0 : c0 + 128].reshape(128, 1))
        db1v = np.ascontiguousarray(
            np.concatenate([db[c0 + 128 : c0 + 192], np.zeros(64, np.float32)]).reshape(
                128, 1
            )
        )
        maps.append({"xT": xTs[b], "wT": wTc, "db0": db0v, "db1": db1v})
    return maps


def kernel(x, W, decay_bias, _trace=False):
    x = np.asarray(x, np.float32)
    W = np.asarray(W, np.float32)
    db = np.asarray(decay_bias, np.float32)
    B, S, _ = x.shape
    nc = _build(S)
    res = run_bass_kernel_spmd(nc, _in_maps(x, W, db), list(range(NCORE)), trace=_trace)
    outf = np.empty((B, S, D_REC), np.float32)
    for core in range(NCORE):
        b, h = core // 2, core % 2
        outf[b, :, h * CH : (h + 1) * CH] = res.results[core]["out"].T
    if _trace:
        return outf, res
    return outf



# revision 2
# speedup vs baseline: 1.0628x; 1.0628x over previous
"""Griffin recurrence Trainium2 kernel.

Sharding: 8 cores = 4 batches x 2 channel-halves (192 channels each).
Layout on device: [channels, seq]. The projection matmul runs on the PE in
float32r (full PE rate at >=256 moving cols); all transcendentals use ONLY
the ln/exp activation table (sigma(z) = exp(-ln(1+exp(-z))), sqrt(x) =
exp(0.5 ln x), 1/clip(cd,1e-10) = exp(min(cum, 23.0259))), so the ACT
engine never swaps function tables. The chunked scan's cross-chunk pass is
stitched incrementally per 512-column block with AP-seeded scans, and the
incoming chunk state is folded into the chunk-start element of the w
sequence before the second scan, so there is no separate combine phase.

Per core:
  inputs  xb  [8*128, 16*512]  x[b].T packed block-major (see _pack_x)
          wT  [2048, 640]      packed W slice (see _pack_w), fp32r-rounded
          nbA [128, 1]         NEGATED decay bias, channels 0..127
          nbB [128, 1]         [-db for channels 128..191; zeros(64)]
  output  out [192, seq]       states, channel-major
"""

import sys

sys.path.insert(0, "/opt/trn_rl_repo")

from contextlib import ExitStack

import numpy as np

from concourse import bacc, mybir, tile
from concourse.bass_utils import run_bass_kernel_spmd

f32 = mybir.dt.float32
f32r = mybir.dt.float32r
AF = mybir.ActivationFunctionType
ALU = mybir.AluOpType
AXL = mybir.AxisListType

D_MODEL = 2048
D_REC = 384
CHUNK = 64
NCORE = 8
CH = 192  # channels per core
BLK = 512  # seq columns per pipeline block (= 1 PSUM bank)
NK = D_MODEL // 128  # 16 k-tiles
NT = 5  # M-tiles per core (640 = 5*128 packed W rows)
CPB = BLK // CHUNK  # chunks per block (8)
LN_EPS = 23.025850929940457  # -ln(1e-10)

_built = {}


def _round_fp32r(a: np.ndarray) -> np.ndarray:
    """Round fp32 to fp32r (11 mantissa bits, low 12 bits zero), RNE."""
    u = np.ascontiguousarray(a, dtype=np.float32).view(np.uint32)
    rem = u & np.uint32(0xFFF)
    keep = u & np.uint32(0xFFFFF000)
    lsb = (u >> np.uint32(12)) & np.uint32(1)
    up = (rem > 0x800) | ((rem == 0x800) & (lsb == 1))
    return (keep + (up.astype(np.uint32) << np.uint32(12))).view(np.float32)


def _emit(tc, nc, xb, wT, nbA, nbB, out, seq):
    nblk = seq // BLK

    with ExitStack() as ctx:
        const = ctx.enter_context(tc.tile_pool(name="const", bufs=1))
        carry = ctx.enter_context(tc.tile_pool(name="carry", bufs=1))
        xp = ctx.enter_context(tc.tile_pool(name="xp", bufs=2))
        pp = ctx.enter_context(tc.tile_pool(name="pp", bufs=1, space="PSUM"))
        wk = ctx.enter_context(tc.tile_pool(name="wk", bufs=1))
        ob_pool = ctx.enter_context(tc.tile_pool(name="obp", bufs=2))

        # constants
        mask = const.tile([128, BLK], f32, tag="mask")
        nc.vector.memset(mask[:], 1.0)
        for c in range(CPB):
            nc.vector.memset(mask[:, c * CHUNK : c * CHUNK + 1], 0.0)
        ones8 = const.tile([128, CPB], f32, tag="ones8")
        nc.vector.memset(ones8[:], 1.0)
        nbA_t = const.tile([128, 1], f32, tag="nbA")
        nc.scalar.dma_start(nbA_t[:], nbA[:])
        nbB_t = const.tile([128, 1], f32, tag="nbB")
        nc.scalar.dma_start(nbB_t[:], nbB[:])
        wt = []
        for k in range(NK):
            w = const.tile([128, NT * 128], f32r, tag=f"wt{k}")
            nc.scalar.dma_start(w[:], wT[k * 128 : (k + 1) * 128, :])
            wt.append(w)

        # cross-block carried stitch state per group
        st_carry = {}
        for g in ("A", "B"):
            Lc = carry.tile([128, 1], f32, tag=f"Lc{g}")
            nc.vector.memset(Lc[:], 0.0)
            Wc = carry.tile([128, 1], f32, tag=f"Wc{g}")
            nc.vector.memset(Wc[:], 0.0)
            IS = carry.tile([128, 1], f32, tag=f"IS{g}")
            nc.vector.memset(IS[:], 0.0)
            st_carry[g] = (Lc, Wc, IS)

        for b in range(nblk):
            s0 = b * BLK
            # x block: fully contiguous 4 MB DMA (host packed block-major)
            xall = xp.tile([128, NK * BLK], f32r, tag="xall")
            nc.sync.dma_start(xall[:], xb[b * 128 : (b + 1) * 128, :])

            ps = []
            for t in range(NT):
                p = pp.tile([128, BLK], f32, tag=f"ps{t}")
                for k in range(NK):
                    nc.tensor.matmul(
                        p[:],
                        wt[k][:, t * 128 : (t + 1) * 128],
                        xall[:, k * BLK : (k + 1) * BLK],
                        start=(k == 0),
                        stop=(k == NK - 1),
                    )
                ps.append(p)

            # PSUM evacuation, ln/exp table only (Copy is table-free)
            ea = wk.tile([128, BLK], f32, tag="ea")
            nc.scalar.activation(ea[:], ps[0][:], AF.Exp, bias=nbA_t[:], scale=-1.0)
            ma = wk.tile([128, BLK], f32, tag="ma")
            nc.scalar.activation(ma[:], ea[:], AF.Ln, bias=1.0)
            ei = wk.tile([128, BLK], f32, tag="ei")
            nc.scalar.activation(ei[:], ps[1][:], AF.Exp, scale=-1.0)
            mi = wk.tile([128, BLK], f32, tag="mi")
            nc.scalar.activation(mi[:], ei[:], AF.Ln, bias=1.0)
            vA = wk.tile([128, BLK], f32, tag="vA")
            nc.scalar.activation(vA[:], ps[2][:], AF.Copy)
            eb = wk.tile([128, BLK], f32, tag="eb")
            nc.scalar.activation(eb[:], ps[3][:], AF.Exp, bias=nbB_t[:], scale=-1.0)
            mb = wk.tile([128, BLK], f32, tag="mb")
            nc.scalar.activation(mb[:], eb[:], AF.Ln, bias=1.0)
            vB = wk.tile([64, BLK], f32, tag="vB")
            nc.scalar.activation(vB[:], ps[4][0:64, :], AF.Copy)
            # realign i_B's m to partitions 0..63
            mbi = wk.tile([64, BLK], f32, tag="mbi")
            nc.scalar.dma_start(mbi[:], mb[64:128, :])

            groups = (
                ("A", 128, ma, mi, vA, 0),
                ("B", 64, mb, mbi, vB, 128),
            )

            for name, pg, m_t, mi_t, v_t, orow in groups:
                m_ap = m_t[0:pg, :]
                Lc, Wc, IS = st_carry[name]

                # intra-chunk cumulative -log-decay
                cum = wk.tile([pg, BLK], f32, tag=f"cum{name}")
                nc.vector.tensor_tensor_scan(
                    cum[:], mask[0:pg, :], m_ap, 0.0, ALU.mult, ALU.add
                )
                a2 = wk.tile([pg, BLK], f32, tag=f"a2{name}")
                nc.scalar.activation(a2[:], m_ap, AF.Exp, scale=-2.0)
                cd = wk.tile([pg, BLK], f32, tag=f"cd{name}")
                nc.scalar.activation(cd[:], cum[:], AF.Exp, scale=-1.0)
                cmin = wk.tile([pg, BLK], f32, tag=f"cmin{name}")
                nc.vector.tensor_scalar_min(cmin[:], cum[:], LN_EPS)
                inv = wk.tile([pg, BLK], f32, tag=f"inv{name}")
                nc.scalar.activation(inv[:], cmin[:], AF.Exp)
                l1 = wk.tile([pg, BLK], f32, tag=f"l1{name}")
                nc.scalar.activation(l1[:], a2[:], AF.Ln, bias=1.0, scale=-1.0)
                st = wk.tile([pg, BLK], f32, tag=f"st{name}")
                nc.vector.scalar_tensor_tensor(
                    st[:], l1[:], 0.5, mi_t[0:pg, :], ALU.mult, ALU.subtract
                )
                sqti = wk.tile([pg, BLK], f32, tag=f"sqti{name}")
                nc.scalar.activation(sqti[:], st[:], AF.Exp)
                u = wk.tile([pg, BLK], f32, tag=f"u{name}")
                nc.gpsimd.tensor_mul(u[:], sqti[:], v_t[0:pg, :])
                w_ = wk.tile([pg, BLK], f32, tag=f"w{name}")
                nc.gpsimd.tensor_mul(w_[:], u[:], inv[:])

                # per-chunk sums of w (pre-fold) -> chunk final states
                wsum = wk.tile([pg, CPB], f32, tag=f"wsum{name}")
                nc.vector.tensor_reduce(
                    wsum[:],
                    w_[:].rearrange("p (c s) -> p c s", c=CPB),
                    AXL.X,
                    ALU.add,
                )
                # incremental cross-chunk stitch
                Mc8 = wk.tile([pg, CPB], f32, tag=f"Mc8{name}")
                nc.vector.tensor_scalar_min(
                    Mc8[:], cum[:, CHUNK - 1 :: CHUNK], LN_EPS
                )
                Lam8 = wk.tile([pg, CPB], f32, tag=f"Lam8{name}")
                nc.vector.tensor_tensor_scan(
                    Lam8[:], ones8[0:pg, :], Mc8[:], Lc[0:pg, :], ALU.mult, ALU.add
                )
                nc.vector.tensor_copy(Lc[0:pg, :], Lam8[:, CPB - 1 : CPB])
                CD8 = wk.tile([pg, CPB], f32, tag=f"CD8{name}")
                nc.scalar.activation(CD8[:], Lam8[:], AF.Exp, scale=-1.0)
                LamC8 = wk.tile([pg, CPB], f32, tag=f"LamC8{name}")
                nc.vector.tensor_scalar_min(LamC8[:], Lam8[:], LN_EPS)
                iCD8 = wk.tile([pg, CPB], f32, tag=f"iCD8{name}")
                nc.scalar.activation(iCD8[:], LamC8[:], AF.Exp)
                F8 = wk.tile([pg, CPB], f32, tag=f"F8{name}")
                nc.vector.tensor_mul(F8[:], cd[:, CHUNK - 1 :: CHUNK], wsum[:])
                t8 = wk.tile([pg, CPB], f32, tag=f"t8{name}")
                nc.vector.tensor_mul(t8[:], F8[:], iCD8[:])
                CW8 = wk.tile([pg, CPB], f32, tag=f"CW8{name}")
                nc.vector.tensor_tensor_scan(
                    CW8[:], ones8[0:pg, :], t8[:], Wc[0:pg, :], ALU.mult, ALU.add
                )
                nc.vector.tensor_copy(Wc[0:pg, :], CW8[:, CPB - 1 : CPB])
                inc8 = wk.tile([pg, CPB], f32, tag=f"inc8{name}")
                nc.vector.tensor_copy(inc8[:, 0:1], IS[0:pg, :])
                nc.vector.tensor_mul(
                    inc8[:, 1:CPB], CD8[:, 0 : CPB - 1], CW8[:, 0 : CPB - 1]
                )
                nc.vector.tensor_mul(
                    IS[0:pg, :], CD8[:, CPB - 1 : CPB], CW8[:, CPB - 1 : CPB]
                )
                # fold incoming state into chunk-start w, then scan
                nc.vector.tensor_add(w_[:, 0::CHUNK], w_[:, 0::CHUNK], inc8[:])
                sw = wk.tile([pg, BLK], f32, tag=f"sw{name}")
                nc.vector.tensor_tensor_scan(
                    sw[:], mask[0:pg, :], w_[:], 0.0, ALU.mult, ALU.add
                )
                ob = ob_pool.tile([pg, BLK], f32, tag=f"ob{name}")
                nc.gpsimd.tensor_mul(ob[:], cd[:], sw[:])
                nc.gpsimd.dma_start(out[orow : orow + pg, s0 : s0 + BLK], ob[:])


def _build(seq):
    if seq in _built:
        return _built[seq]
    nc = bacc.Bacc(
        "TRN2", target_bir_lowering=False, debug=False, num_devices=NCORE
    )
    nblk = seq // BLK
    xb = nc.dram_tensor(
        "xb", [nblk * 128, NK * BLK], f32r, kind="ExternalInput"
    ).ap()
    wT = nc.dram_tensor("wT", [D_MODEL, NT * 128], f32r, kind="ExternalInput").ap()
    nbA = nc.dram_tensor("nbA", [128, 1], f32, kind="ExternalInput").ap()
    nbB = nc.dram_tensor("nbB", [128, 1], f32, kind="ExternalInput").ap()
    out = nc.dram_tensor("out", [CH, seq], f32, kind="ExternalOutput").ap()
    with tile.TileContext(nc) as tc:
        _emit(tc, nc, xb, wT, nbA, nbB, out, seq)
    nc.compile()
    _built[seq] = nc
    return nc


def _pack_w(W, h):
    """Pack this half's W rows into 640 rows of 5 M-tiles.

    t0 = a[0:128], t1 = i[0:128], t2 = v[0:128],
    t3 = [a[128:192]; i[128:192]], t4 = [v[128:192]; zeros]."""
    c0 = h * CH
    z = np.zeros((64, W.shape[1]), np.float32)
    return np.concatenate(
        [
            W[c0 : c0 + 128],
            W[D_REC + c0 : D_REC + c0 + 128],
            W[2 * D_REC + c0 : 2 * D_REC + c0 + 128],
            W[c0 + 128 : c0 + 192],
            W[D_REC + c0 + 128 : D_REC + c0 + 192],
            W[2 * D_REC + c0 + 128 : 2 * D_REC + c0 + 192],
            z,
        ],
        axis=0,
    )


def _pack_x(xb):
    """x[b] [seq, 2048] -> [nblk*128, 16*512] block-major fp32r."""
    seq = xb.shape[0]
    nblk = seq // BLK
    xr = xb.reshape(nblk, BLK, NK, 128).transpose(0, 3, 2, 1)
    return _round_fp32r(np.ascontiguousarray(xr).reshape(nblk * 128, NK * BLK))


def _in_maps(x, W, db):
    maps = []
    xbs = {}
    for core in range(NCORE):
        b, h = core // 2, core % 2
        if b not in xbs:
            xbs[b] = _pack_x(x[b])
        c0 = h * CH
        wTc = _round_fp32r(np.ascontiguousarray(_pack_w(W, h).T))
        nbAv = np.ascontiguousarray((-db[c0 : c0 + 128]).reshape(128, 1))
        nbBv = np.ascontiguousarray(
            np.concatenate(
                [-db[c0 + 128 : c0 + 192], np.zeros(64, np.float32)]
            ).reshape(128, 1)
        )
        maps.append({"xb": xbs[b], "wT": wTc, "nbA": nbAv, "nbB": nbBv})
    return maps


def kernel(x, W, decay_bias, _trace=False):
    x = np.asarray(x, np.float32)
    W = np.asarray(W, np.float32)
    db = np.asarray(decay_bias, np.float32)
    B, S, _ = x.shape
    nc = _build(S)
    res = run_bass_kernel_spmd(nc, _in_maps(x, W, db), list(range(NCORE)), trace=_trace)
    outf = np.empty((B, S, D_REC), np.float32)
    for core in range(NCORE):
        b, h = core // 2, core % 2
        outf[b, :, h * CH : (h + 1) * CH] = res.results[core]["out"].T
    if _trace:
        return outf, res
    return outf


# revision 5
# speedup vs baseline: 1.3329x; 1.2541x over previous
"""Griffin recurrence Trainium2 kernel.

Sharding: 8 cores = 4 batches x 2 channel-halves (192 channels each).
Layout on device: [channels, seq]. The projection matmul runs on the PE in
float32r (full PE rate at >=256 moving cols); all transcendentals use ONLY
the ln/exp activation table (sigma(z) = exp(-ln(1+exp(-z))), sqrt(x) =
exp(0.5 ln x), 1/clip(cd,1e-10) = exp(min(cum, 23.0259))), so the ACT
engine never swaps function tables. The chunked scan's cross-chunk pass is
stitched incrementally per 512-column block with AP-seeded scans, and the
incoming chunk state is folded into the chunk-start element of the w
sequence before the second scan, so there is no separate combine phase.

Per core:
  inputs  xb  [8*128, 16*512]  x[b].T packed block-major (see _pack_x)
          wT  [2048, 640]      packed W slice (see _pack_w), fp32r-rounded
          nbA [128, 1]         NEGATED decay bias, channels 0..127
          nbB [128, 1]         [-db for channels 128..191; zeros(64)]
  output  out [192, seq]       states, channel-major
"""

import sys

sys.path.insert(0, "/opt/trn_rl_repo")

from contextlib import ExitStack

import numpy as np

import bass_rust as _bass_rust

from concourse import bacc, mybir, tile
from concourse.bass_utils import run_bass_kernel_spmd
from concourse.hw_specs import get_activation_tables

f32 = mybir.dt.float32
f32r = mybir.dt.float32r
AF = mybir.ActivationFunctionType
ALU = mybir.AluOpType
AXL = mybir.AxisListType

D_MODEL = 2048
D_REC = 384
CHUNK = 64
NCORE = 8
CH = 192  # channels per core
BLK = 512  # seq columns per pipeline block (= 1 PSUM bank)
NK = D_MODEL // 128  # 16 k-tiles
NT = 5  # M-tiles per core (640 = 5*128 packed W rows)
CPB = BLK // CHUNK  # chunks per block (8)
LN_EPS = 23.025850929940457  # -ln(1e-10)

_built = {}


class _Bacc(bacc.Bacc):
    """Bacc whose activation-table chooser is restricted to the one table
    holding every function this kernel uses (exp, ln, copy), so the ACT
    engine performs a single table load instead of swapping per call.
    Table list positions are preserved — `act_func_set_id` indexes
    act_info.json — only the candidate function sets are masked."""

    _ACT_TABLE = "natural_log_exp_and_others"

    def insert_act_table_loads(self):
        has_activation = any(
            isinstance(i, mybir.InstActivation)
            for b in self.main_func.blocks
            for i in b.instructions
        )
        if not has_activation:
            return
        tables = [
            (name, funcs if name == self._ACT_TABLE else set())
            for name, funcs in get_activation_tables(self.m.arch).items()
        ]
        _bass_rust.insert_act_table_loads(self, tables)


def _round_fp32r(a: np.ndarray) -> np.ndarray:
    """Round fp32 to fp32r (11 mantissa bits, low 12 bits zero), RNE."""
    u = np.ascontiguousarray(a, dtype=np.float32).view(np.uint32)
    rem = u & np.uint32(0xFFF)
    keep = u & np.uint32(0xFFFFF000)
    lsb = (u >> np.uint32(12)) & np.uint32(1)
    up = (rem > 0x800) | ((rem == 0x800) & (lsb == 1))
    return (keep + (up.astype(np.uint32) << np.uint32(12))).view(np.float32)


def _emit(tc, nc, xb, wT, nbA, nbB, out, seq):
    nblk = seq // BLK

    with ExitStack() as ctx:
        const = ctx.enter_context(tc.tile_pool(name="const", bufs=1))
        carry = ctx.enter_context(tc.tile_pool(name="carry", bufs=1))
        xp = ctx.enter_context(tc.tile_pool(name="xp", bufs=2))
        pp = ctx.enter_context(tc.tile_pool(name="pp", bufs=1, space="PSUM"))
        wk = ctx.enter_context(tc.tile_pool(name="wk", bufs=1))
        ob_pool = ctx.enter_context(tc.tile_pool(name="obp", bufs=2))

        # constants
        mask = const.tile([128, BLK], f32, tag="mask")
        nc.vector.memset(mask[:], 1.0)
        for c in range(CPB):
            nc.vector.memset(mask[:, c * CHUNK : c * CHUNK + 1], 0.0)
        ones8 = const.tile([128, CPB], f32, tag="ones8")
        nc.vector.memset(ones8[:], 1.0)
        nbA_t = const.tile([128, 1], f32, tag="nbA")
        nc.scalar.dma_start(nbA_t[:], nbA[:])
        nbB_t = const.tile([128, 1], f32, tag="nbB")
        nc.scalar.dma_start(nbB_t[:], nbB[:])
        wt = []
        for k in range(NK):
            w = const.tile([128, NT * 128], f32r, tag=f"wt{k}")
            nc.scalar.dma_start(w[:], wT[k * 128 : (k + 1) * 128, :])
            wt.append(w)

        # cross-block carried stitch state per group
        st_carry = {}
        for g in ("A", "B"):
            Lc = carry.tile([128, 1], f32, tag=f"Lc{g}")
            nc.vector.memset(Lc[:], 0.0)
            Wc = carry.tile([128, 1], f32, tag=f"Wc{g}")
            nc.vector.memset(Wc[:], 0.0)
            IS = carry.tile([128, 1], f32, tag=f"IS{g}")
            nc.vector.memset(IS[:], 0.0)
            st_carry[g] = (Lc, Wc, IS)

        for b in range(nblk):
            s0 = b * BLK
            # x block: fully contiguous 4 MB DMA (host packed block-major)
            xall = xp.tile([128, NK * BLK], f32r, tag="xall")
            nc.sync.dma_start(xall[:], xb[b * 128 : (b + 1) * 128, :])

            ps = []
            for t in range(NT):
                p = pp.tile([128, BLK], f32, tag=f"ps{t}")
                for k in range(NK):
                    nc.tensor.matmul(
                        p[:],
                        wt[k][:, t * 128 : (t + 1) * 128],
                        xall[:, k * BLK : (k + 1) * BLK],
                        start=(k == 0),
                        stop=(k == NK - 1),
                    )
                ps.append(p)

            # PSUM evacuation, ln/exp table only (Copy is table-free)
            ea = wk.tile([128, BLK], f32, tag="ea")
            nc.scalar.activation(ea[:], ps[0][:], AF.Exp, bias=nbA_t[:], scale=-1.0)
            ma = wk.tile([128, BLK], f32, tag="ma")
            nc.scalar.activation(ma[:], ea[:], AF.Ln, bias=1.0)
            ei = wk.tile([128, BLK], f32, tag="ei")
            nc.scalar.activation(ei[:], ps[1][:], AF.Exp, scale=-1.0)
            mi = wk.tile([128, BLK], f32, tag="mi")
            nc.scalar.activation(mi[:], ei[:], AF.Ln, bias=1.0)
            vA = wk.tile([128, BLK], f32, tag="vA")
            nc.scalar.activation(vA[:], ps[2][:], AF.Copy)
            eb = wk.tile([128, BLK], f32, tag="eb")
            nc.scalar.activation(eb[:], ps[3][:], AF.Exp, bias=nbB_t[:], scale=-1.0)
            mb = wk.tile([128, BLK], f32, tag="mb")
            nc.scalar.activation(mb[:], eb[:], AF.Ln, bias=1.0)
            vB = wk.tile([64, BLK], f32, tag="vB")
            nc.scalar.activation(vB[:], ps[4][0:64, :], AF.Copy)
            # realign i_B's m to partitions 0..63
            mbi = wk.tile([64, BLK], f32, tag="mbi")
            nc.scalar.dma_start(mbi[:], mb[64:128, :])

            groups = (
                ("A", 128, ma, mi, vA, 0),
                ("B", 64, mb, mbi, vB, 128),
            )

            for name, pg, m_t, mi_t, v_t, orow in groups:
                m_ap = m_t[0:pg, :]
                Lc, Wc, IS = st_carry[name]

                # intra-chunk cumulative -log-decay
                cum = wk.tile([pg, BLK], f32, tag=f"cum{name}")
                nc.vector.tensor_tensor_scan(
                    cum[:], mask[0:pg, :], m_ap, 0.0, ALU.mult, ALU.add
                )
                a2 = wk.tile([pg, BLK], f32, tag=f"a2{name}")
                nc.scalar.activation(a2[:], m_ap, AF.Exp, scale=-2.0)
                cd = wk.tile([pg, BLK], f32, tag=f"cd{name}")
                nc.scalar.activation(cd[:], cum[:], AF.Exp, scale=-1.0)
                cmin = wk.tile([pg, BLK], f32, tag=f"cmin{name}")
                nc.vector.tensor_scalar_min(cmin[:], cum[:], LN_EPS)
                inv = wk.tile([pg, BLK], f32, tag=f"inv{name}")
                nc.scalar.activation(inv[:], cmin[:], AF.Exp)
                l1 = wk.tile([pg, BLK], f32, tag=f"l1{name}")
                nc.scalar.activation(l1[:], a2[:], AF.Ln, bias=1.0, scale=-1.0)
                st = wk.tile([pg, BLK], f32, tag=f"st{name}")
                nc.vector.scalar_tensor_tensor(
                    st[:], l1[:], 0.5, mi_t[0:pg, :], ALU.mult, ALU.subtract
                )
                sqti = wk.tile([pg, BLK], f32, tag=f"sqti{name}")
                nc.scalar.activation(sqti[:], st[:], AF.Exp)
                u = wk.tile([pg, BLK], f32, tag=f"u{name}")
                nc.gpsimd.tensor_mul(u[:], sqti[:], v_t[0:pg, :])
                w_ = wk.tile([pg, BLK], f32, tag=f"w{name}")
                nc.gpsimd.tensor_mul(w_[:], u[:], inv[:])

                # per-chunk sums of w (pre-fold) -> chunk final states
                wsum = wk.tile([pg, CPB], f32, tag=f"wsum{name}")
                nc.vector.tensor_reduce(
                    wsum[:],
                    w_[:].rearrange("p (c s) -> p c s", c=CPB),
                    AXL.X,
                    ALU.add,
                )
                # incremental cross-chunk stitch
                Mc8 = wk.tile([pg, CPB], f32, tag=f"Mc8{name}")
                nc.vector.tensor_scalar_min(
                    Mc8[:], cum[:, CHUNK - 1 :: CHUNK], LN_EPS
                )
                Lam8 = wk.tile([pg, CPB], f32, tag=f"Lam8{name}")
                nc.vector.tensor_tensor_scan(
                    Lam8[:], ones8[0:pg, :], Mc8[:], Lc[0:pg, :], ALU.mult, ALU.add
                )
                nc.vector.tensor_copy(Lc[0:pg, :], Lam8[:, CPB - 1 : CPB])
                CD8 = wk.tile([pg, CPB], f32, tag=f"CD8{name}")
                nc.scalar.activation(CD8[:], Lam8[:], AF.Exp, scale=-1.0)
                LamC8 = wk.tile([pg, CPB], f32, tag=f"LamC8{name}")
                nc.vector.tensor_scalar_min(LamC8[:], Lam8[:], LN_EPS)
                iCD8 = wk.tile([pg, CPB], f32, tag=f"iCD8{name}")
                nc.scalar.activation(iCD8[:], LamC8[:], AF.Exp)
                F8 = wk.tile([pg, CPB], f32, tag=f"F8{name}")
                nc.vector.tensor_mul(F8[:], cd[:, CHUNK - 1 :: CHUNK], wsum[:])
                t8 = wk.tile([pg, CPB], f32, tag=f"t8{name}")
                nc.vector.tensor_mul(t8[:], F8[:], iCD8[:])
                CW8 = wk.tile([pg, CPB], f32, tag=f"CW8{name}")
                nc.vector.tensor_tensor_scan(
                    CW8[:], ones8[0:pg, :], t8[:], Wc[0:pg, :], ALU.mult, ALU.add
                )
                nc.vector.tensor_copy(Wc[0:pg, :], CW8[:, CPB - 1 : CPB])
                inc8 = wk.tile([pg, CPB], f32, tag=f"inc8{name}")
                nc.vector.tensor_copy(inc8[:, 0:1], IS[0:pg, :])
                nc.vector.tensor_mul(
                    inc8[:, 1:CPB], CD8[:, 0 : CPB - 1], CW8[:, 0 : CPB - 1]
                )
                nc.vector.tensor_mul(
                    IS[0:pg, :], CD8[:, CPB - 1 : CPB], CW8[:, CPB - 1 : CPB]
                )
                # fold incoming state into chunk-start w, then scan
                nc.vector.tensor_add(w_[:, 0::CHUNK], w_[:, 0::CHUNK], inc8[:])
                sw = wk.tile([pg, BLK], f32, tag=f"sw{name}")
                nc.vector.tensor_tensor_scan(
                    sw[:], mask[0:pg, :], w_[:], 0.0, ALU.mult, ALU.add
                )
                ob = ob_pool.tile([pg, BLK], f32, tag=f"ob{name}")
                nc.gpsimd.tensor_mul(ob[:], cd[:], sw[:])
                nc.gpsimd.dma_start(out[orow : orow + pg, s0 : s0 + BLK], ob[:])


def _build(seq):
    if seq in _built:
        return _built[seq]
    nc = _Bacc(
        "TRN2", target_bir_lowering=False, debug=False, num_devices=NCORE
    )
    nblk = seq // BLK
    xb = nc.dram_tensor(
        "xb", [nblk * 128, NK * BLK], f32r, kind="ExternalInput"
    ).ap()
    wT = nc.dram_tensor("wT", [D_MODEL, NT * 128], f32r, kind="ExternalInput").ap()
    nbA = nc.dram_tensor("nbA", [128, 1], f32, kind="ExternalInput").ap()
    nbB = nc.dram_tensor("nbB", [128, 1], f32, kind="ExternalInput").ap()
    out = nc.dram_tensor("out", [CH, seq], f32, kind="ExternalOutput").ap()
    with tile.TileContext(nc) as tc:
        _emit(tc, nc, xb, wT, nbA, nbB, out, seq)
    nc.compile()
    _built[seq] = nc
    return nc


def _pack_w(W, h):
    """Pack this half's W rows into 640 rows of 5 M-tiles.

    t0 = a[0:128], t1 = i[0:128], t2 = v[0:128],
    t3 = [a[128:192]; i[128:192]], t4 = [v[128:192]; zeros]."""
    c0 = h * CH
    z = np.zeros((64, W.shape[1]), np.float32)
    return np.concatenate(
        [
            W[c0 : c0 + 128],
            W[D_REC + c0 : D_REC + c0 + 128],
            W[2 * D_REC + c0 : 2 * D_REC + c0 + 128],
            W[c0 + 128 : c0 + 192],
            W[D_REC + c0 + 128 : D_REC + c0 + 192],
            W[2 * D_REC + c0 + 128 : 2 * D_REC + c0 + 192],
            z,
        ],
        axis=0,
    )


def _pack_x(xb):
    """x[b] [seq, 2048] -> [nblk*128, 16*512] block-major fp32r."""
    seq = xb.shape[0]
    nblk = seq // BLK
    xr = xb.reshape(nblk, BLK, NK, 128).transpose(0, 3, 2, 1)
    return _round_fp32r(np.ascontiguousarray(xr).reshape(nblk * 128, NK * BLK))


def _in_maps(x, W, db):
    maps = []
    xbs = {}
    for core in range(NCORE):
        b, h = core // 2, core % 2
        if b not in xbs:
            xbs[b] = _pack_x(x[b])
        c0 = h * CH
        wTc = _round_fp32r(np.ascontiguousarray(_pack_w(W, h).T))
        nbAv = np.ascontiguousarray((-db[c0 : c0 + 128]).reshape(128, 1))
        nbBv = np.ascontiguousarray(
            np.concatenate(
                [-db[c0 + 128 : c0 + 192], np.zeros(64, np.float32)]
            ).reshape(128, 1)
        )
        maps.append({"xb": xbs[b], "wT": wTc, "nbA": nbAv, "nbB": nbBv})
    return maps


def kernel(x, W, decay_bias, _trace=False):
    x = np.asarray(x, np.float32)
    W = np.asarray(W, np.float32)
    db = np.asarray(decay_bias, np.float32)
    B, S, _ = x.shape
    nc = _build(S)
    res = run_bass_kernel_spmd(nc, _in_maps(x, W, db), list(range(NCORE)), trace=_trace)
    outf = np.empty((B, S, D_REC), np.float32)
    for core in range(NCORE):
        b, h = core // 2, core % 2
        outf[b, :, h * CH : (h + 1) * CH] = res.results[core]["out"].T
    if _trace:
        return outf, res
    return outf


# revision 8
# speedup vs baseline: 1.3965x; 1.0478x over previous
"""Griffin recurrence Trainium2 kernel.

Sharding: 8 cores = 4 batches x 2 channel-halves (192 channels each).
Layout on device: [channels, seq]. The projection matmul runs on the PE in
float32r (full PE rate at >=256 moving cols); all transcendentals use ONLY
the ln/exp activation table (sigma(z) = exp(-ln(1+exp(-z))), sqrt(x) =
exp(0.5 ln x), 1/clip(cd,1e-10) = exp(min(cum, 23.0259))), so the ACT
engine never swaps function tables. The chunked scan's cross-chunk pass is
stitched incrementally per 512-column block with AP-seeded scans, and the
incoming chunk state is folded into the chunk-start element of the w
sequence before the second scan, so there is no separate combine phase.

Per core:
  inputs  xb  [8*128, 16*512]  x[b].T packed block-major (see _pack_x)
          wT  [2048, 640]      packed W slice (see _pack_w), fp32r-rounded
          nbA [128, 1]         NEGATED decay bias, channels 0..127
          nbB [128, 1]         [-db for channels 128..191; zeros(64)]
  output  out [192, seq]       states, channel-major
"""

import sys

sys.path.insert(0, "/opt/trn_rl_repo")

from contextlib import ExitStack

import numpy as np

import bass_rust as _bass_rust

from concourse import bacc, mybir, tile
from concourse.bass_utils import run_bass_kernel_spmd
from concourse.hw_specs import get_activation_tables

f32 = mybir.dt.float32
f32r = mybir.dt.float32r
AF = mybir.ActivationFunctionType
ALU = mybir.AluOpType
AXL = mybir.AxisListType

D_MODEL = 2048
D_REC = 384
CHUNK = 64
NCORE = 8
CH = 192  # channels per core
BLK = 512  # seq columns per pipeline block (= 1 PSUM bank)
NK = D_MODEL // 128  # 16 k-tiles
NT = 5  # M-tiles per core (640 = 5*128 packed W rows)
CPB = BLK // CHUNK  # chunks per block (8)
LN_EPS = 23.025850929940457  # -ln(1e-10)

_built = {}


class _Bacc(bacc.Bacc):
    """Bacc whose activation-table chooser is restricted to the one table
    holding every function this kernel uses (exp, ln, copy), so the ACT
    engine performs a single table load instead of swapping per call.
    Table list positions are preserved — `act_func_set_id` indexes
    act_info.json — only the candidate function sets are masked."""

    _ACT_TABLE = "natural_log_exp_and_others"

    def insert_act_table_loads(self):
        has_activation = any(
            isinstance(i, mybir.InstActivation)
            for b in self.main_func.blocks
            for i in b.instructions
        )
        if not has_activation:
            return
        tables = [
            (name, funcs if name == self._ACT_TABLE else set())
            for name, funcs in get_activation_tables(self.m.arch).items()
        ]
        _bass_rust.insert_act_table_loads(self, tables)


def _round_fp32r(a: np.ndarray) -> np.ndarray:
    """Round fp32 to fp32r (11 mantissa bits, low 12 bits zero), RNE."""
    u = np.ascontiguousarray(a, dtype=np.float32).view(np.uint32)
    rem = u & np.uint32(0xFFF)
    keep = u & np.uint32(0xFFFFF000)
    lsb = (u >> np.uint32(12)) & np.uint32(1)
    up = (rem > 0x800) | ((rem == 0x800) & (lsb == 1))
    return (keep + (up.astype(np.uint32) << np.uint32(12))).view(np.float32)


def _emit(tc, nc, xb, wT, nbA, nbB, out, seq):
    nblk = seq // BLK

    with ExitStack() as ctx:
        const = ctx.enter_context(tc.tile_pool(name="const", bufs=1))
        carry = ctx.enter_context(tc.tile_pool(name="carry", bufs=1))
        xp = ctx.enter_context(tc.tile_pool(name="xp", bufs=2))
        pp = ctx.enter_context(tc.tile_pool(name="pp", bufs=1, space="PSUM"))
        wk = ctx.enter_context(tc.tile_pool(name="wk", bufs=1))
        ob_pool = ctx.enter_context(tc.tile_pool(name="obp", bufs=2))

        # constants
        mask = const.tile([128, BLK], f32, tag="mask")
        nc.vector.memset(mask[:], 1.0)
        for c in range(CPB):
            nc.vector.memset(mask[:, c * CHUNK : c * CHUNK + 1], 0.0)
        ones8 = const.tile([128, CPB], f32, tag="ones8")
        nc.vector.memset(ones8[:], 1.0)
        nbA_t = const.tile([128, 1], f32, tag="nbA")
        nc.scalar.dma_start(nbA_t[:], nbA[:])
        nbB_t = const.tile([128, 1], f32, tag="nbB")
        nc.scalar.dma_start(nbB_t[:], nbB[:])
        # weight tiles spread across queues so early matmuls aren't starved
        wq = (nc.scalar, nc.gpsimd)
        wt = []
        for k in range(NK):
            w = const.tile([128, NT * 128], f32r, tag=f"wt{k}")
            wq[k % 2].dma_start(w[:], wT[k * 128 : (k + 1) * 128, :])
            wt.append(w)

        # cross-block carried stitch state per group
        st_carry = {}
        for g in ("A", "B"):
            Lc = carry.tile([128, 1], f32, tag=f"Lc{g}")
            nc.vector.memset(Lc[:], 0.0)
            Wc = carry.tile([128, 1], f32, tag=f"Wc{g}")
            nc.vector.memset(Wc[:], 0.0)
            IS = carry.tile([128, 1], f32, tag=f"IS{g}")
            nc.vector.memset(IS[:], 0.0)
            st_carry[g] = (Lc, Wc, IS)

        def emit_block(s0, w):
            """One pipeline step covering seq columns [s0, s0+w). w <= BLK,
            w a multiple of CHUNK; tiles are allocated at BLK width and
            sliced so sub-width steps reuse the same pool buffers."""
            cpb = w // CHUNK
            blk = s0 // BLK
            c0 = s0 - blk * BLK  # column offset inside the packed x block
            xk = []
            for k in range(NK):
                xt = xp.tile([128, BLK], f32r, tag=f"xk{k}")
                nc.sync.dma_start(
                    xt[:, 0:w],
                    xb[blk * 128 : (blk + 1) * 128, k * BLK + c0 : k * BLK + c0 + w],
                )
                xk.append(xt)

            ps = []
            for t in range(NT):
                p = pp.tile([128, BLK], f32, tag=f"ps{t}")
                for k in range(NK):
                    nc.tensor.matmul(
                        p[:, 0:w],
                        wt[k][:, t * 128 : (t + 1) * 128],
                        xk[k][:, 0:w],
                        start=(k == 0),
                        stop=(k == NK - 1),
                    )
                ps.append(p)

            # PSUM evacuation, ln/exp table only (Copy is table-free)
            ea = wk.tile([128, BLK], f32, tag="ea")
            nc.scalar.activation(
                ea[:, 0:w], ps[0][:, 0:w], AF.Exp, bias=nbA_t[:], scale=-1.0
            )
            ma = wk.tile([128, BLK], f32, tag="ma")
            nc.scalar.activation(ma[:, 0:w], ea[:, 0:w], AF.Ln, bias=1.0)
            ei = wk.tile([128, BLK], f32, tag="ei")
            nc.scalar.activation(ei[:, 0:w], ps[1][:, 0:w], AF.Exp, scale=-1.0)
            mi = wk.tile([128, BLK], f32, tag="mi")
            nc.scalar.activation(mi[:, 0:w], ei[:, 0:w], AF.Ln, bias=1.0)
            vA = wk.tile([128, BLK], f32, tag="vA")
            nc.scalar.activation(vA[:, 0:w], ps[2][:, 0:w], AF.Copy)
            eb = wk.tile([128, BLK], f32, tag="eb")
            nc.scalar.activation(
                eb[:, 0:w], ps[3][:, 0:w], AF.Exp, bias=nbB_t[:], scale=-1.0
            )
            mb = wk.tile([128, BLK], f32, tag="mb")
            nc.scalar.activation(mb[:, 0:w], eb[:, 0:w], AF.Ln, bias=1.0)
            vB = wk.tile([64, BLK], f32, tag="vB")
            nc.scalar.activation(vB[:, 0:w], ps[4][0:64, 0:w], AF.Copy)
            # realign i_B's m to partitions 0..63
            mbi = wk.tile([64, BLK], f32, tag="mbi")
            nc.scalar.dma_start(mbi[:, 0:w], mb[64:128, 0:w])

            groups = (
                ("A", 128, ma, mi, vA, 0),
                ("B", 64, mb, mbi, vB, 128),
            )

            for name, pg, m_t, mi_t, v_t, orow in groups:
                m_ap = m_t[0:pg, 0:w]
                Lc, Wc, IS = st_carry[name]

                # intra-chunk cumulative -log-decay
                cum = wk.tile([pg, BLK], f32, tag=f"cum{name}")
                nc.vector.tensor_tensor_scan(
                    cum[:, 0:w], mask[0:pg, 0:w], m_ap, 0.0, ALU.mult, ALU.add
                )
                a2 = wk.tile([pg, BLK], f32, tag=f"a2{name}")
                nc.scalar.activation(a2[:, 0:w], m_ap, AF.Exp, scale=-2.0)
                cd = wk.tile([pg, BLK], f32, tag=f"cd{name}")
                nc.scalar.activation(cd[:, 0:w], cum[:, 0:w], AF.Exp, scale=-1.0)
                cmin = wk.tile([pg, BLK], f32, tag=f"cmin{name}")
                nc.vector.tensor_scalar_min(cmin[:, 0:w], cum[:, 0:w], LN_EPS)
                inv = wk.tile([pg, BLK], f32, tag=f"inv{name}")
                nc.scalar.activation(inv[:, 0:w], cmin[:, 0:w], AF.Exp)
                l1 = wk.tile([pg, BLK], f32, tag=f"l1{name}")
                nc.scalar.activation(
                    l1[:, 0:w], a2[:, 0:w], AF.Ln, bias=1.0, scale=-1.0
                )
                st = wk.tile([pg, BLK], f32, tag=f"st{name}")
                nc.vector.scalar_tensor_tensor(
                    st[:, 0:w], l1[:, 0:w], 0.5, mi_t[0:pg, 0:w],
                    ALU.mult, ALU.subtract,
                )
                sqti = wk.tile([pg, BLK], f32, tag=f"sqti{name}")
                nc.scalar.activation(sqti[:, 0:w], st[:, 0:w], AF.Exp)
                u = wk.tile([pg, BLK], f32, tag=f"u{name}")
                nc.gpsimd.tensor_mul(u[:, 0:w], sqti[:, 0:w], v_t[0:pg, 0:w])
                w_ = wk.tile([pg, BLK], f32, tag=f"w{name}")
                nc.gpsimd.tensor_mul(w_[:, 0:w], u[:, 0:w], inv[:, 0:w])

                # per-chunk sums of w (pre-fold) -> chunk final states
                wsum = wk.tile([pg, CPB], f32, tag=f"wsum{name}")
                nc.vector.tensor_reduce(
                    wsum[:, 0:cpb],
                    w_[:, 0:w].rearrange("p (c s) -> p c s", c=cpb),
                    AXL.X,
                    ALU.add,
                )
                # incremental cross-chunk stitch
                Mc8 = wk.tile([pg, CPB], f32, tag=f"Mc8{name}")
                nc.vector.tensor_scalar_min(
                    Mc8[:, 0:cpb], cum[:, CHUNK - 1 : w : CHUNK], LN_EPS
                )
                Lam8 = wk.tile([pg, CPB], f32, tag=f"Lam8{name}")
                nc.vector.tensor_tensor_scan(
                    Lam8[:, 0:cpb], ones8[0:pg, 0:cpb], Mc8[:, 0:cpb],
                    Lc[0:pg, :], ALU.mult, ALU.add,
                )
                nc.vector.tensor_copy(Lc[0:pg, :], Lam8[:, cpb - 1 : cpb])
                CD8 = wk.tile([pg, CPB], f32, tag=f"CD8{name}")
                nc.scalar.activation(
                    CD8[:, 0:cpb], Lam8[:, 0:cpb], AF.Exp, scale=-1.0
                )
                LamC8 = wk.tile([pg, CPB], f32, tag=f"LamC8{name}")
                nc.vector.tensor_scalar_min(LamC8[:, 0:cpb], Lam8[:, 0:cpb], LN_EPS)
                iCD8 = wk.tile([pg, CPB], f32, tag=f"iCD8{name}")
                nc.scalar.activation(iCD8[:, 0:cpb], LamC8[:, 0:cpb], AF.Exp)
                F8 = wk.tile([pg, CPB], f32, tag=f"F8{name}")
                nc.vector.tensor_mul(
                    F8[:, 0:cpb], cd[:, CHUNK - 1 : w : CHUNK], wsum[:, 0:cpb]
                )
                t8 = wk.tile([pg, CPB], f32, tag=f"t8{name}")
                nc.vector.tensor_mul(t8[:, 0:cpb], F8[:, 0:cpb], iCD8[:, 0:cpb])
                CW8 = wk.tile([pg, CPB], f32, tag=f"CW8{name}")
                nc.vector.tensor_tensor_scan(
                    CW8[:, 0:cpb], ones8[0:pg, 0:cpb], t8[:, 0:cpb],
                    Wc[0:pg, :], ALU.mult, ALU.add,
                )
                nc.vector.tensor_copy(Wc[0:pg, :], CW8[:, cpb - 1 : cpb])
                inc8 = wk.tile([pg, CPB], f32, tag=f"inc8{name}")
                nc.vector.tensor_copy(inc8[:, 0:1], IS[0:pg, :])
                if cpb > 1:
                    nc.vector.tensor_mul(
                        inc8[:, 1:cpb], CD8[:, 0 : cpb - 1], CW8[:, 0 : cpb - 1]
                    )
                nc.vector.tensor_mul(
                    IS[0:pg, :], CD8[:, cpb - 1 : cpb], CW8[:, cpb - 1 : cpb]
                )
                # fold incoming state into chunk-start w, then scan
                nc.vector.tensor_add(
                    w_[:, 0:w:CHUNK], w_[:, 0:w:CHUNK], inc8[:, 0:cpb]
                )
                sw = wk.tile([pg, BLK], f32, tag=f"sw{name}")
                nc.vector.tensor_tensor_scan(
                    sw[:, 0:w], mask[0:pg, 0:w], w_[:, 0:w], 0.0, ALU.mult, ALU.add
                )
                ob = ob_pool.tile([pg, BLK], f32, tag=f"ob{name}")
                nc.gpsimd.tensor_mul(ob[:, 0:w], cd[:, 0:w], sw[:, 0:w])
                nc.gpsimd.dma_start(out[orow : orow + pg, s0 : s0 + w], ob[:, 0:w])

        # full blocks, then the final block in two half-width steps so the
        # post-matmul chain tail after the last matmul is short
        for b in range(nblk - 1):
            emit_block(b * BLK, BLK)
        emit_block((nblk - 1) * BLK, BLK // 2)
        emit_block((nblk - 1) * BLK + BLK // 2, BLK // 2)


def _build(seq):
    if seq in _built:
        return _built[seq]
    nc = _Bacc(
        "TRN2", target_bir_lowering=False, debug=False, num_devices=NCORE
    )
    nblk = seq // BLK
    xb = nc.dram_tensor(
        "xb", [nblk * 128, NK * BLK], f32r, kind="ExternalInput"
    ).ap()
    wT = nc.dram_tensor("wT", [D_MODEL, NT * 128], f32r, kind="ExternalInput").ap()
    nbA = nc.dram_tensor("nbA", [128, 1], f32, kind="ExternalInput").ap()
    nbB = nc.dram_tensor("nbB", [128, 1], f32, kind="ExternalInput").ap()
    out = nc.dram_tensor("out", [CH, seq], f32, kind="ExternalOutput").ap()
    with tile.TileContext(nc) as tc:
        _emit(tc, nc, xb, wT, nbA, nbB, out, seq)
    nc.compile()
    _built[seq] = nc
    return nc


def _pack_w(W, h):
    """Pack this half's W rows into 640 rows of 5 M-tiles.

    t0 = a[0:128], t1 = i[0:128], t2 = v[0:128],
    t3 = [a[128:192]; i[128:192]], t4 = [v[128:192]; zeros]."""
    c0 = h * CH
    z = np.zeros((64, W.shape[1]), np.float32)
    return np.concatenate(
        [
            W[c0 : c0 + 128],
            W[D_REC + c0 : D_REC + c0 + 128],
            W[2 * D_REC + c0 : 2 * D_REC + c0 + 128],
            W[c0 + 128 : c0 + 192],
            W[D_REC + c0 + 128 : D_REC + c0 + 192],
            W[2 * D_REC + c0 + 128 : 2 * D_REC + c0 + 192],
            z,
        ],
        axis=0,
    )


def _pack_x(xb):
    """x[b] [seq, 2048] -> [nblk*128, 16*512] block-major fp32r."""
    seq = xb.shape[0]
    nblk = seq // BLK
    xr = xb.reshape(nblk, BLK, NK, 128).transpose(0, 3, 2, 1)
    return _round_fp32r(np.ascontiguousarray(xr).reshape(nblk * 128, NK * BLK))


def _in_maps(x, W, db):
    maps = []
    xbs = {}
    for core in range(NCORE):
        b, h = core // 2, core % 2
        if b not in xbs:
            xbs[b] = _pack_x(x[b])
        c0 = h * CH
        wTc = _round_fp32r(np.ascontiguousarray(_pack_w(W, h).T))
        nbAv = np.ascontiguousarray((-db[c0 : c0 + 128]).reshape(128, 1))
        nbBv = np.ascontiguousarray(
            np.concatenate(
                [-db[c0 + 128 : c0 + 192], np.zeros(64, np.float32)]
            ).reshape(128, 1)
        )
        maps.append({"xb": xbs[b], "wT": wTc, "nbA": nbAv, "nbB": nbBv})
    return maps


def kernel(x, W, decay_bias, _trace=False):
    x = np.asarray(x, np.float32)
    W = np.asarray(W, np.float32)
    db = np.asarray(decay_bias, np.float32)
    B, S, _ = x.shape
    nc = _build(S)
    res = run_bass_kernel_spmd(nc, _in_maps(x, W, db), list(range(NCORE)), trace=_trace)
    outf = np.empty((B, S, D_REC), np.float32)
    for core in range(NCORE):
        b, h = core // 2, core % 2
        outf[b, :, h * CH : (h + 1) * CH] = res.results[core]["out"].T
    if _trace:
        return outf, res
    return outf
